# revision 64
# baseline (speedup 1.0000x reference)
"""Trainium2 Bass kernel for nn_AttentionLayer (dense transformer block).

Reference computation (B=16, S=1024, F=512, H=8, DH=64):
    q/k/v = einsum('bsf,hfd->hbsd', x, w{q,k,v})
    att   = softmax over the BATCH axis of (q @ k^T / sqrt(DH))
    out   = att @ v  -> concat heads -> @ w_out + b_out -> LayerNorm -> LeakyReLU(0.1)

Sharding: one head per core (8 heads, 8 cores). Softmax over batch is
fully local to a head, so the only communication is an AllToAll that
redistributes per-head attention outputs into per-token-slice columns
before the output projection. Core i computes output tokens
[2048*i, 2048*(i+1)) = batches (2i, 2i+1); the host concatenates.

Optimizations over the f32 baseline (663us -> 282us cost-model):
- x is converted to bf16 on the host: half the HBM bytes, and the DMA
  crossbar transposes x straight into SBUF (dma_start_transpose), so
  phase A has no PE transposes, no PSUM staging, no cast copies.
- bf16 matmuls everywhere (f32 att@v was 4 cycles/row; bf16 is 1).
- Packed q|k stationary: one projection matmul emits q^T in PSUM
  partitions 0:64 and k^T in 64:128; partition-shifted copies split.
- v computed in natural [token, dh] layout (x^T tile as stationary).
- bf16 softmax element-ops (2x DVE perf mode), balanced across
  DVE (tree + rec + 2 muls) and Pool (2 muls), exp on Act.
- Phase B is one continuous software pipeline over all (scp, tcn)
  passes; att@v runs 4 passes behind the scores/softmax chain and its
  halves interleave with score groups so the in-order PE queue never
  serializes the loop-carried softmax chain.
- bf16 AllToAll in four sc-pair chunks: three overlap phase B, only
  the last sits on the tail; phase C consumes chunks in arrival order.
- PE-side bias broadcast add (ones[1,128]^T @ bias[1,F] accumulated
  into the projection PSUM group), LN affine fused into one Act
  Identity(scale=rstd, bias=-mu*rstd) op reading PSUM directly.
- Pinned warm matmuls bridge the last-collective wait so the final
  output-projection tiles run at the peak PE p-state.

Self-contained: hardcodes all shapes; no sibling imports.
"""

import json

import ml_dtypes
import numpy as np

import concourse.bass as bass
import concourse.tile as tile
from concourse import mybir
from concourse.bass_utils import run_bass_kernel_spmd
from concourse.tile_rust import add_dep_helper

F32 = mybir.dt.float32
F32R = mybir.dt.float32r
BF16 = mybir.dt.bfloat16

B, S, F, H, DH = 16, 1024, 512, 8, 64
NT = B * S            # 16384 tokens total
NCORES = 8
TPC = NT // NCORES    # 2048 tokens per core (= 2 batches)
NEG_SLOPE = 0.1
LN_EPS = 1e-5
INV_SQRT_DH = 1.0 / 8.0
NKT = F // 128        # 4 k-tiles over input features
NCH = 32              # phase-A chunks of 512 tokens
SC = 128              # softmax s-chunk
NSC = S // SC         # 8 s-chunks
NTC = S // 128        # 8 t-chunks


# --------------------------------------------------------------------------
# BIR post-fix: this container's walrus encodes at most ONE sem wait per
# instruction. Split any multi-wait instruction by inserting single-wait
# Drains before it on the same engine.
# --------------------------------------------------------------------------
def _split_multi_waits(raw: bytes) -> bytes:
    m = json.loads(raw)
    ctr = 0
    changed = False
    for fn in m["functions"]:
        for bb in fn["blocks"]:
            out = []
            for inst in bb["instructions"]:
                si = inst.get("sync_info")
                ow = (si or {}).get("on_wait") or []
                if si and len(ow) > 1:
                    changed = True
                    for w in ow[:-1]:
                        ctr += 1
                        out.append({
                            "name": f"WFIX-{ctr}",
                            "opcode": "Drain",
                            "engine": inst["engine"],
                            "ins": [], "outs": [],
                            "sync_info": {"on_wait": [w], "on_update": []},
                        })
                    si["on_wait"] = ow[-1:]
                out.append(inst)
            bb["instructions"] = out
    return json.dumps(m).encode() if changed else raw


def _install_birfix(nc):
    orig = nc.to_json_bytes
    nc.to_json_bytes = lambda: _split_multi_waits(orig())


def _bcast_free(ap: bass.AP, count: int) -> bass.AP:
    """[P, N] -> [P, count, N] with the middle dim broadcast (step 0)."""
    return bass.AP(tensor=ap.tensor, offset=ap.offset,
                   ap=[ap.ap[0], [0, count], ap.ap[1]])


def _bcast_part(ap: bass.AP, parts: int) -> bass.AP:
    """[N] (1-D dram) -> [parts, N] broadcast across partitions."""
    return bass.AP(tensor=ap.tensor, offset=ap.offset,
                   ap=[[0, parts]] + list(ap.ap))


def _one_part(ap: bass.AP) -> bass.AP:
    """[N] (1-D dram) -> [1, N]."""
    return bass.AP(tensor=ap.tensor, offset=ap.offset,
                   ap=[[0, 1]] + list(ap.ap))


# --------------------------------------------------------------------------
# Kernel program (SPMD; identical on all cores, per-head weights as inputs)
# --------------------------------------------------------------------------
def build_nc(has_gamma: bool, has_beta: bool, dbg: bool = False,
             phases: str = "ABC"):
    nc = bass.Bass("TRN2", target_bir_lowering=False, debug=False,
                   num_devices=NCORES)

    x_d = nc.declare_dram_parameter("x", [NT, F], BF16, isOutput=False)
    wq_d = nc.declare_dram_parameter("wq", [F, DH], F32, isOutput=False)
    wk_d = nc.declare_dram_parameter("wk", [F, DH], F32, isOutput=False)
    wv_d = nc.declare_dram_parameter("wv", [F, DH], F32, isOutput=False)
    wout_d = nc.declare_dram_parameter("wout", [F, F], F32, isOutput=False)
    bout_d = nc.declare_dram_parameter("bout", [F], F32, isOutput=False)
    gamma_d = beta_d = None
    if has_gamma:
        gamma_d = nc.declare_dram_parameter("gamma", [F], F32, isOutput=False)
    if has_beta:
        beta_d = nc.declare_dram_parameter("beta", [F], F32, isOutput=False)
    y_d = nc.declare_dram_parameter("y", [TPC, F], F32, isOutput=True)

    # AllToAll split into four chunks (sc pairs) so early collectives
    # overlap phase B and only the last sc pair is on the tail.
    a2a_in = [nc.dram_tensor(f"a2a_in{q}", [NCORES, DH, TPC // 4], BF16)
              for q in range(4)]
    a2a_out = [nc.dram_tensor(f"a2a_out{q}", [NCORES, DH, TPC // 4], BF16)
               for q in range(4)]

    with tile.TileContext(nc) as tc:
        with (
            tc.tile_pool(name="consts", bufs=1) as consts,
            tc.tile_pool(name="persist", bufs=1) as persist,
        ):
            # packed q|k stationary: cols 0:64 = wq ktile, 64:128 = wk ktile
            wqk_sb = consts.tile([128, NKT, 2 * DH], BF16)
            wv_sb = consts.tile([128, NKT, DH], BF16)
            wout_sb = consts.tile([128, NKT, F], BF16)
            with tc.tile_pool(name="wstage", bufs=1) as stgp:
                wstg = stgp.tile([128, NKT, 2 * DH], F32)
                nc.sync.dma_start(
                    out=wstg[:, :, 0:DH],
                    in_=wq_d.ap().rearrange("(j p) d -> p j d", p=128))
                nc.sync.dma_start(
                    out=wstg[:, :, DH:2 * DH],
                    in_=wk_d.ap().rearrange("(j p) d -> p j d", p=128))
                nc.vector.tensor_copy(wqk_sb, wstg)
                wvstg = stgp.tile([128, NKT, DH], F32)
                nc.sync.dma_start(
                    out=wvstg,
                    in_=wv_d.ap().rearrange("(j p) d -> p j d", p=128))
                nc.vector.tensor_copy(wv_sb, wvstg)
                wout_stg = stgp.tile([128, NKT, F], F32)
                nc.sync.dma_start(
                    out=wout_stg,
                    in_=wout_d.ap().rearrange("(j p) n -> p j n", p=128))
                nc.vector.tensor_copy(wout_sb, wout_stg)
                bias_stg = stgp.tile([1, F], F32)
                nc.sync.dma_start(out=bias_stg, in_=_one_part(bout_d.ap()))
                bias_row = consts.tile([1, F], BF16)
                nc.vector.tensor_copy(bias_row, bias_stg)
            ones1 = consts.tile([1, 128], BF16)
            nc.vector.memset(ones1, 1.0)
            gamma_bc = beta_bc = None
            if has_gamma:
                gamma_bc = consts.tile([128, F], F32)
                nc.sync.dma_start(out=gamma_bc,
                                  in_=_bcast_part(gamma_d.ap(), 128))
            if has_beta:
                beta_bc = consts.tile([128, F], F32)
                nc.sync.dma_start(out=beta_bc,
                                  in_=_bcast_part(beta_d.ap(), 128))
            eps_sb = consts.tile([128, 1], F32)
            nc.vector.memset(eps_sb, LN_EPS)

            # token-split transposed q/k: col c, rows 0:64  = tokens c (< 8192)
            #                             col c, rows 64:128 = tokens c + 8192
            # (scores matmul needs lhsT and rhs at the SAME base partition;
            #  batch b < 8 reads rows 0:64 of both, b >= 8 reads rows 64:128)
            qT2 = persist.tile([128, NT // 2], BF16)
            kT2 = persist.tile([128, NT // 2], BF16)
            # v natural: block t holds v[128*t : 128*(t+1), :] as [128, 64]
            v_all = persist.tile([128, (NT // 128) * DH], BF16)

            a2a_dma_insts = []

            # -------------- Phase A: x^T, q/k/v projections ---------------
            if "A" not in phases:
                pass
            else:
             with (
                tc.tile_pool(name="pa_xt", bufs=3) as xtpool,
                tc.tile_pool(name="pa_ps_qk", bufs=4, space="PSUM") as ps_qk,
                tc.tile_pool(name="pa_ps_v", bufs=4, space="PSUM") as ps_v,
            ):
                x_ap = x_d.ap()
                GT = 1024          # tokens per transpose group (8 chunks)
                for g8 in range(NT // GT):
                    # x arrives bf16; the DMA crossbar transposes straight
                    # into SBUF (no PE transposes, no PSUM staging)
                    xts8 = []
                    for j in range(NKT):
                        xt8 = xtpool.tile([128, GT], BF16, tag=f"xt{j}",
                                          name=f"xt_{g8}_{j}")
                        nc.sync.dma_start_transpose(
                            xt8,
                            x_ap[g8 * GT:(g8 + 1) * GT,
                                 j * 128:(j + 1) * 128])
                        xts8.append(xt8)
                    for c8 in range(GT // 512):
                        c = g8 * (GT // 512) + c8
                        half = 0 if c < NCH // 2 else 64
                        cc = c % (NCH // 2)
                        xts = [xts8[j][:, c8 * 512:(c8 + 1) * 512]
                               for j in range(NKT)]
                        # one matmul computes q^T (parts 0:64) and
                        # k^T (64:128)
                        pqk = ps_qk.tile([128, 512], F32, tag="pqk")
                        for j in range(NKT):
                            nc.tensor.matmul(pqk, wqk_sb[:, j, :], xts[j],
                                             start=(j == 0),
                                             stop=(j == NKT - 1))
                        nc.scalar.copy(
                            out=qT2[half:half + DH, cc * 512:(cc + 1) * 512],
                            in_=pqk[0:DH, :])
                        nc.vector.tensor_copy(
                            kT2[half:half + DH, cc * 512:(cc + 1) * 512],
                            pqk[DH:128, :])
                        # v in natural [token, dh] layout: x^T is stationary
                        pv4 = ps_v.tile([128, 512], F32, tag="pv4")
                        for a in range(4):
                            for j in range(NKT):
                                nc.tensor.matmul(
                                    pv4[:, a * DH:(a + 1) * DH],
                                    xts[j][:, a * 128:(a + 1) * 128],
                                    wv_sb[:, j, :],
                                    start=(a == 0 and j == 0),
                                    stop=(a == 3 and j == NKT - 1),
                                    skip_group_check=True)
                        nc.vector.tensor_copy(
                            v_all[:, c * 4 * DH:(c + 1) * 4 * DH],
                            pv4[:, 0:4 * DH])

            # -------------- Phase B: attention ----------------------------
            if "B" not in phases:
                pass
            else:
             with (
                tc.tile_pool(name="pb_e", bufs=16) as epool,
                tc.tile_pool(name="pb_en", bufs=24) as enpool,
                tc.tile_pool(name="pb_den", bufs=6) as denpool,
                tc.tile_pool(name="pb_rec", bufs=8) as recpool,
                tc.tile_pool(name="pb_ot", bufs=4) as otpool,
                tc.tile_pool(name="pb_ps_s", bufs=2, space="PSUM") as ps_s,
                tc.tile_pool(name="pb_ps_o", bufs=4, space="PSUM") as ps_o,
            ):
                SP = 2 * SC   # scores computed over s-pairs (N=256)
                G_OF = {0: 0, 8: 1, 4: 2, 12: 3}   # batch-group -> tile idx
                NPASS = (NSC // 2) * NTC + 4
                en_hist = {}
                po_tiles = {}
                DELAY = 4

                def emit_scores_exp(scp, tcn, g, b0, e_g):
                    psc = ps_s.tile([128, 4 * SP], F32, tag="psc",
                                    name=f"ps_{scp}_{tcn}_{g}")
                    half = 0 if b0 < 8 else 64
                    for bi4 in range(4):
                        b = b0 + bi4
                        bb = b % 8
                        lhsT = kT2[half:half + DH,
                                   bb * S + tcn * 128:
                                   bb * S + (tcn + 1) * 128]
                        rhs = qT2[half:half + DH,
                                  bb * S + scp * SP:
                                  bb * S + (scp + 1) * SP]
                        nc.tensor.matmul(
                            psc[:, bi4 * SP:(bi4 + 1) * SP],
                            lhsT, rhs,
                            start=(bi4 % 2 == 0),
                            stop=(bi4 % 2 == 1),
                            skip_group_check=True)
                    e_t = epool.tile([128, 4 * SP], BF16, tag="e",
                                     name=f"e_{scp}_{tcn}_{g}")
                    nc.scalar.activation(
                        out=e_t, in_=psc,
                        func=mybir.ActivationFunctionType.Exp,
                        scale=INV_SQRT_DH)
                    e_g.append(e_t)

                def emit_attv_half(scp_p, tp, scl, pp_en):
                    po_pairs = po_tiles[scp_p]
                    for bg in range(4):
                        p = bg % 2
                        rb_ = 64 * (bg // 2)
                        for bi in range(4):
                            b = bg * 4 + bi
                            g = G_OF[(b // 4) * 4]
                            t128 = b * (S // 128) + tp
                            # one start per (bank, partition-half)
                            nc.tensor.matmul(
                                po_pairs[scl][p][rb_:rb_ + DH,
                                                 bi * SC:(bi + 1) * SC],
                                v_all[:, t128 * DH:(t128 + 1) * DH],
                                pp_en[g][:, (b % 4) * SP + scl * SC:
                                         (b % 4) * SP + (scl + 1) * SC],
                                start=(tp == 0 and bi == 0),
                                stop=(tp == NTC - 1 and bi == 3),
                                skip_group_check=True)

                for k in range(NPASS):
                    live = k < (NSC // 2) * NTC
                    attv = k >= DELAY
                    if attv:
                        scp_p, tp = divmod(k - DELAY, NTC)
                        if tp == 0:
                            po_tiles[scp_p] = [
                                [ps_o.tile([128, 4 * SC], F32, tag="po",
                                           name=f"po_{scp_p}_{l}_{g}")
                                 for g in range(2)] for l in range(2)]
                        pp_en = en_hist.pop(k - DELAY)
                    if live:
                        scp, tcn = divmod(k, NTC)
                        e_g = []
                        emit_scores_exp(scp, tcn, 0, 0, e_g)
                        emit_scores_exp(scp, tcn, 1, 8, e_g)
                    # att@v half 1 interleaves with the next score groups so
                    # the PE's in-order queue keeps Act fed
                    if attv:
                        emit_attv_half(scp_p, tp, 0, pp_en)
                    if live:
                        emit_scores_exp(scp, tcn, 2, 4, e_g)
                        emit_scores_exp(scp, tcn, 3, 12, e_g)
                        # denominator tree across group tiles:
                        # groups hold b {0-3}, {8-11}, {4-7}, {12-15}
                        t1a = denpool.tile([128, 4 * SP], BF16, tag="t1a")
                        nc.vector.tensor_add(t1a, e_g[0], e_g[1])
                        t1b = denpool.tile([128, 4 * SP], BF16, tag="t1b")
                        nc.vector.tensor_add(t1b, e_g[2], e_g[3])
                        t2 = denpool.tile([128, 4 * SP], BF16, tag="t2")
                        nc.vector.tensor_add(t2, t1a, t1b)
                        t3 = denpool.tile([128, 2 * SP], BF16, tag="t3")
                        nc.vector.tensor_add(t3, t2[:, 0:2 * SP],
                                             t2[:, 2 * SP:4 * SP])
                        den = denpool.tile([128, SP], BF16, tag="den")
                        nc.vector.tensor_add(den, t3[:, 0:SP],
                                             t3[:, SP:2 * SP])
                        rec = recpool.tile([128, SP], BF16, tag="rec")
                        with nc.allow_low_precision(reason="bf16 softmax"):
                            nc.vector.reciprocal(rec, den)
                        en_g = []
                        for g in range(4):
                            en = enpool.tile([128, 4 * SP], BF16, tag="en",
                                             name=f"en_{scp}_{tcn}_{g}")
                            # near the end of B, Pool's serial mul backlog
                            # would gate the att@v drain and the final
                            # collective; shift those muls to DVE
                            if g >= 2 and k < (NSC // 2) * NTC - 4:
                                nc.gpsimd.tensor_mul(
                                    en, e_g[g], _bcast_free(rec[:, :], 4))
                            else:
                                nc.vector.tensor_mul(
                                    en, e_g[g], _bcast_free(rec[:, :], 4))
                            en_g.append(en)
                        en_hist[k] = en_g
                    if attv:
                        emit_attv_half(scp_p, tp, 1, pp_en)
                        if tp == NTC - 1:
                            po_pairs = po_tiles[scp_p]
                            for scl in range(2):
                                sc = scp_p * 2 + scl
                                oT_sc = otpool.tile([128, 8 * SC], BF16,
                                                    tag="ot",
                                                    name=f"ot_{sc}")
                                nc.vector.tensor_copy(
                                    oT_sc[:, 0:4 * SC],
                                    po_pairs[scl][0])
                                nc.vector.tensor_copy(
                                    oT_sc[:, 4 * SC:8 * SC],
                                    po_pairs[scl][1])
                                q = sc // 2
                                scl2 = sc % 2
                                HT = TPC // 4
                                for hh in range(2):
                                    for bp in range(2):
                                        sb = oT_sc[hh * 64:(hh + 1) * 64, :]
                                        src_ = bass.AP(
                                            tensor=sb.tensor,
                                            offset=sb.offset + bp * SC,
                                            ap=[list(sb.ap[0]), [2 * SC, 4],
                                                [1, SC]])
                                        ins = nc.sync.dma_start(
                                            out=bass.AP(
                                                tensor=a2a_in[q].ap().tensor,
                                                offset=hh * 4 * DH * HT
                                                + bp * (S // 4) + scl2 * SC,
                                                ap=[[HT, DH], [DH * HT, 4],
                                                    [1, SC]]),
                                            in_=src_)
                                        a2a_dma_insts.append((q, ins))

            # -------------- AllToAll --------------------------------------
            if "C" not in phases:
                cc = None
            else:
             cc = [nc.gpsimd.collective_compute(
                 "AllToAll", mybir.AluOpType.bypass,
                 replica_groups=[list(range(NCORES))],
                 ins=[a2a_in[q].ap()], outs=[a2a_out[q].ap()])
                   for q in range(4)]
            if cc is not None:
                for q, di in a2a_dma_insts:
                    add_dep_helper(cc[q].ins, di.ins,
                                   reason="a2a waits for oT stores")

            # -------------- Phase C: out-proj + bias + LN + LeakyReLU -----
            with (
                tc.tile_pool(name="pc_cat", bufs=1) as catpool,
                tc.tile_pool(name="pc_y", bufs=4) as ypool,
                tc.tile_pool(name="pc_st", bufs=6) as stpool,
                tc.tile_pool(name="pc_ps_y", bufs=3, space="PSUM") as ps_y,
                tc.tile_pool(name="pc_ps_w", bufs=1, space="PSUM") as ps_w,
            ):
                cat_sb = catpool.tile([128, NKT, TPC], BF16)
                for q in range(4):
                    for bp in range(2):
                        rb = nc.sync.dma_start(
                            out=cat_sb[:, :,
                                       bp * S + q * (S // 4):
                                       bp * S + q * (S // 4) + S // 4],
                            in_=a2a_out[q].ap()[:, :,
                                                bp * (S // 4):
                                                (bp + 1) * (S // 4)]
                            .rearrange("(j a) d t -> (a d) j t", j=4))
                        if cc is not None:
                            add_dep_helper(rb.ins, cc[q].ins,
                                           reason="readback waits for a2a")
                # m-tile order follows a2a chunk arrival so only the last
                # sc pair's output projection sits on the tail
                warm = ps_w.tile([1, F], F32)
                for m in (0, 1, 8, 9, 2, 3, 10, 11, 4, 5, 12, 13,
                          -1, 6, 7, 14, 15):
                    if m < 0:
                        # matmuls pinned behind the 12th tile's rstd bridge
                        # the last-collective wait so the PE p-state stays
                        # at peak for the final output-projection tiles
                        pin1 = stpool.tile([128, 1], BF16, tag="pin1")
                        nc.vector.tensor_copy(pin1, rstd)
                        for w in range(70):
                            nc.tensor.matmul(warm, pin1, wout_sb[:, 0, :],
                                             start=True, stop=True)
                        continue
                    py = ps_y.tile([128, F], F32, tag="py")
                    for j in range(NKT):
                        nc.tensor.matmul(
                            py, cat_sb[:, j, m * 128:(m + 1) * 128],
                            wout_sb[:, j, :],
                            start=(j == 0), stop=False)
                    # bias broadcast add on the PE: ones[1,128]^T @ bias[1,F]
                    nc.tensor.matmul(py, ones1, bias_row,
                                     start=False, stop=True)
                    stats = stpool.tile([128, 6], F32, tag="stats")
                    nc.vector.bn_stats(out=stats, in_=py)
                    mv = stpool.tile([128, 2], F32, tag="mv")
                    nc.vector.bn_aggr(out=mv, in_=stats)
                    rstd = stpool.tile([128, 1], F32, tag="rstd")
                    nc.scalar.activation(
                        out=rstd, in_=mv[:, 1:2],
                        func=mybir.ActivationFunctionType.Sqrt, bias=eps_sb)
                    nc.vector.reciprocal(rstd, rstd)
                    if not has_gamma and not has_beta:
                        # Identity(rstd*y - mu*rstd) == LN(y) on the Act
                        # engine (Lrelu's alpha is ignored by this walrus,
                        # so leaky stays on Pool)
                        nb = stpool.tile([128, 1], F32, tag="nb")
                        nc.vector.scalar_tensor_tensor(
                            out=nb, in0=mv[:, 0:1], scalar=-1.0, in1=rstd,
                            op0=mybir.AluOpType.mult,
                            op1=mybir.AluOpType.mult)
                        y_sb = ypool.tile([128, F], F32, tag="ysb")
                        nc.scalar.activation(
                            out=y_sb, in_=py,
                            func=mybir.ActivationFunctionType.Identity,
                            scale=rstd, bias=nb)
                        yo = ypool.tile([128, F], F32, tag="yo")
                        # LeakyReLU(0.1): max(x, 0.1*x) since 0 < slope < 1
                        nc.vector.scalar_tensor_tensor(
                            out=yo, in0=y_sb, scalar=NEG_SLOPE, in1=y_sb,
                            op0=mybir.AluOpType.mult,
                            op1=mybir.AluOpType.max)
                    else:
                        y_sb = ypool.tile([128, F], F32, tag="ysb")
                        nc.vector.tensor_scalar(out=y_sb, in0=py,
                                                scalar1=mv[:, 0:1],
                                                scalar2=rstd,
                                                op0=mybir.AluOpType.subtract,
                                                op1=mybir.AluOpType.mult)
                        if has_gamma:
                            nc.vector.tensor_mul(y_sb, y_sb, gamma_bc)
                        if has_beta:
                            nc.vector.tensor_add(y_sb, y_sb, beta_bc)
                        yo = ypool.tile([128, F], F32, tag="yo")
                        # LeakyReLU(0.1): max(x, 0.1*x) since 0 < slope < 1
                        nc.vector.scalar_tensor_tensor(
                            out=yo, in0=y_sb, scalar=NEG_SLOPE, in1=y_sb,
                            op0=mybir.AluOpType.mult, op1=mybir.AluOpType.max)
                    nc.sync.dma_start(out=y_d.ap()[m * 128:(m + 1) * 128, :],
                                      in_=yo)

    _install_birfix(nc)
    return nc


_NC_CACHE = {}


def kernel(**inputs) -> np.ndarray:
    x = np.ascontiguousarray(np.asarray(inputs["x"], dtype=np.float32))
    wq = np.asarray(inputs["wq"], dtype=np.float32)
    wk = np.asarray(inputs["wk"], dtype=np.float32)
    wv = np.asarray(inputs["wv"], dtype=np.float32)
    w_out = np.ascontiguousarray(np.asarray(inputs["w_out"], dtype=np.float32))
    b_out = np.asarray(inputs["b_out"], dtype=np.float32)
    gamma = np.asarray(inputs["ln_gamma"], dtype=np.float32)
    beta = np.asarray(inputs["ln_beta"], dtype=np.float32)

    has_gamma = not np.allclose(gamma, 1.0)
    has_beta = bool(np.any(beta))

    key = (has_gamma, has_beta)
    if key not in _NC_CACHE:
        _NC_CACHE[key] = build_nc(has_gamma, has_beta)
    nc = _NC_CACHE[key]

    x2 = np.ascontiguousarray(x.reshape(NT, F)).astype(ml_dtypes.bfloat16)
    in_maps = []
    for i in range(NCORES):
        m = {"x": x2, "wq": np.ascontiguousarray(wq[i]),
             "wk": np.ascontiguousarray(wk[i]),
             "wv": np.ascontiguousarray(wv[i]),
             "wout": w_out, "bout": b_out}
        if has_gamma:
            m["gamma"] = gamma
        if has_beta:
            m["beta"] = beta
        in_maps.append(m)

    res = run_bass_kernel_spmd(nc, in_maps, list(range(NCORES)))
    global LAST_RESULTS
    LAST_RESULTS = res
    y = np.concatenate([res.results[i]["y"] for i in range(NCORES)], axis=0)
    return y.reshape(B, S, F)


LAST_RESULTS = None
